# revision 1
# baseline (speedup 1.0000x reference)
"""Trainium2 Bass kernel for nn_LocalLoadBalancingLoss.

loss = mean_b var_l(u) + 0.5 * mean_b max_l(u),
u[b,l] = (sum_{t: link(t)=l} pred[b,t] * dem[b, t//8]) / (cap[l] + 1e-8)

Strategy (pure data parallel over batch, 8 cores x 8192 rows):
  per 128-row tile:
    - DMA pred [128,792] + dem [128,99] (contiguous)
    - DVE: tt = pred * broadcast(dem)  (one tensor_tensor op, FD=792)
    - PE : 7x transpose of tt chunks -> PSUM, ACT evacuates to SBUF
    - PE : 7x scatter matmul  uT[128b,16l] += ttT_chunk.T @ onehot_mask
           (mask built on host from tunnel_to_link; accumulated in PSUM,
            8 tiles packed per PSUM bank)
  per 8-tile group: scale by 1/cap (host constant), fused DVE stats
  (sum/max/sum-of-squares) into [128, n_groups] accumulators.
  Host: final tiny reduction + combine across 8 cores.
"""

from contextlib import ExitStack

import numpy as np

import concourse.bass as bass
import concourse.tile as tile
from concourse import mybir
from concourse.bass_utils import run_bass_kernel_spmd
from bass_rust import ScopedClock

N_CORES = 8
B, T, D, L = 65536, 792, 99, 16
ROWS = B // N_CORES  # 8192 rows per core
P = 128
NT = ROWS // P  # 64 tiles per core
NCH = (T + P - 1) // P  # 7 tunnel chunks
TPAD = NCH * P  # 896 (792 zero-padded)
G = 8  # tiles per stats group (G*L = 512 f32 = one PSUM bank)
NG = NT // G

F32 = mybir.dt.float32
X = mybir.AxisListType = mybir.AxisListType


class _TileContext(tile.TileContext):
    """Workaround: this walrus build allows only 1 sync-wait per
    instruction; stock TileContext packs one wait per outstanding proc
    onto the single tail drain. Spread them across multiple drains."""

    def _drain_and_barrier(self, tick_clock, wait_clock):
        nc = self.nc
        drain_inst = nc.sync.drain()
        wait_clock.add_sem_waits(
            drain_inst.ins, ScopedClock({None: tick_clock.global_clock})
        )
        si = drain_inst.ins.sync_info
        waits = list(si.on_wait) if si is not None and si.on_wait else []
        if len(waits) > 1:
            drain_inst.ins.sync_info = mybir.SyncInfo(
                on_wait=[waits[0]], on_update=list(si.on_update or [])
            )
            for w in waits[1:]:
                d = nc.sync.drain()
                d.ins.sync_info = mybir.SyncInfo(on_wait=[w], on_update=[])
        nc.all_engine_barrier()
        assert self.sems is not None
        popped = nc._tile_sem_poison_stack.pop()
        assert popped is self._sem_poison
        nc.clear_and_free_semaphores(list(self.sems.allocated().values()))
        nc.all_engine_barrier()


def _split_multi_waits(nc):
    """This walrus build accepts only 1 sync-wait per instruction (2 for
    EventSemaphore). Hoist extra semaphore waits onto same-engine NOPs
    inserted immediately before the instruction (engine queues are strict
    FIFO, so a preceding wait-NOP is semantically identical)."""
    for fn in nc.m.functions:
        for blk in fn.blocks:
            insts = blk.instructions
            out = []
            for inst in insts:
                si = inst.sync_info
                waits = list(si.on_wait) if si is not None and si.on_wait else []
                cap = 2 if isinstance(inst, mybir.InstEventSemaphore) else 1
                if len(waits) > cap and inst.engine != mybir.EngineType.Unassigned:
                    for w in waits[:-1]:
                        nop = mybir.InstNoOp(
                            name=f"{inst.name}-w{len(out)}",
                            engine=inst.engine,
                            sync_info=mybir.SyncInfo(on_wait=[w], on_update=[]),
                            bass_nofuse=True,
                        )
                        nc.register_instruction(nop, overwrite=True)
                        out.append(nop)
                    inst.sync_info = mybir.SyncInfo(
                        on_wait=[waits[-1]], on_update=list(si.on_update or [])
                    )
                out.append(inst)
            blk.instructions = out


def build_kernel(pe_dt=F32, repeat=1, n_tiles=NT, stages=("mul", "trans", "scat", "stats")):
    """Build the per-core Bass module. pe_dt: dtype for tt/ttT/mask/identity
    (PE path). repeat: replicate the whole pass (for timing builds).
    stages: knock out pipeline stages for profiling builds."""
    ng = max(1, n_tiles // G)
    nc = bass.Bass("TRN2", target_bir_lowering=False, debug=False, num_devices=1)
    pred_d = nc.dram_tensor("pred", [ROWS, T], F32, kind="ExternalInput")
    dem_d = nc.dram_tensor("dem", [ROWS, D], F32, kind="ExternalInput")
    mask_d = nc.dram_tensor("mask", [P, NCH * L], pe_dt, kind="ExternalInput")
    ident_d = nc.dram_tensor("ident", [P, P], pe_dt, kind="ExternalInput")
    rrep_d = nc.dram_tensor("rrep", [P, L], F32, kind="ExternalInput")
    out_d = nc.dram_tensor("partials", [3, P, ng], F32, kind="ExternalOutput")

    with _TileContext(nc) as tc:
        with ExitStack() as ctx:
            singles = ctx.enter_context(tc.tile_pool(name="singles", bufs=1))
            io = ctx.enter_context(tc.tile_pool(name="io", bufs=4))
            work = ctx.enter_context(tc.tile_pool(name="work", bufs=3))
            small = ctx.enter_context(tc.tile_pool(name="small", bufs=2))
            tpsum = ctx.enter_context(tc.tile_pool(name="tpsum", bufs=2, space="PSUM"))
            upsum = ctx.enter_context(tc.tile_pool(name="upsum", bufs=2, space="PSUM"))

            ident_t = singles.tile([P, P], pe_dt)
            nc.sync.dma_start(ident_t[:], ident_d.ap())
            mask_t = singles.tile([P, NCH * L], pe_dt)
            nc.sync.dma_start(mask_t[:], mask_d.ap())
            rrep_t = singles.tile([P, L], F32)
            nc.sync.dma_start(rrep_t[:], rrep_d.ap())
            accq = singles.tile([P, ng], F32)
            accs2 = singles.tile([P, ng], F32)
            accm = singles.tile([P, ng], F32)
            if "stats" not in stages:  # profiling builds: keep outputs defined
                for acc in (accq, accs2, accm):
                    nc.gpsimd.memset(acc[:], 0.0)

            for rep in range(repeat):
                for g in range(ng):
                    u_ps = upsum.tile([P, G, L], F32)
                    for j in range(G):
                        i = (g * G + j) % n_tiles
                        pred_t = io.tile([P, T], F32)
                        nc.sync.dma_start(pred_t[:], pred_d.ap()[i * P : (i + 1) * P, :])
                        dem_t = io.tile([P, D], F32)
                        nc.sync.dma_start(dem_t[:], dem_d.ap()[i * P : (i + 1) * P, :])

                        if "mul" not in stages:
                            continue
                        tt = work.tile([P, TPAD], pe_dt)
                        nc.gpsimd.memset(tt[:, T:TPAD], 0.0)
                        nc.vector.tensor_tensor(
                            out=tt[:, 0:T].rearrange("p (d j) -> p d j", j=8),
                            in0=pred_t[:].rearrange("p (d j) -> p d j", j=8),
                            in1=dem_t[:].unsqueeze(2).broadcast_to([P, D, 8]),
                            op=mybir.AluOpType.mult,
                        )
                        if "trans" not in stages:
                            continue
                        ttT_ps = tpsum.tile([P, TPAD], pe_dt)
                        for c in range(NCH):
                            nc.tensor.transpose(
                                out=ttT_ps[:, c * P : (c + 1) * P],
                                in_=tt[:, c * P : (c + 1) * P],
                                identity=ident_t[:],
                            )
                        ttT = work.tile([P, TPAD], pe_dt)
                        nc.scalar.copy(out=ttT[:], in_=ttT_ps[:])
                        if "scat" not in stages:
                            continue
                        for c in range(NCH):
                            nc.tensor.matmul(
                                out=u_ps[:, j, :],
                                lhsT=ttT[:, c * P : (c + 1) * P],
                                rhs=mask_t[:, c * L : (c + 1) * L],
                                start=(c == 0),
                                stop=(c == NCH - 1),
                            )
                    # --- stats for this group of G tiles ---
                    if "stats" not in stages:
                        continue
                    u_sb = work.tile([P, G, L], F32)
                    nc.vector.tensor_tensor(
                        out=u_sb[:],
                        in0=u_ps[:],
                        in1=rrep_t[:].unsqueeze(1).broadcast_to([P, G, L]),
                        op=mybir.AluOpType.mult,
                    )
                    s8 = small.tile([P, G], F32)
                    nc.vector.reduce_sum(out=s8[:], in_=u_sb[:], axis=X.X)
                    m8 = small.tile([P, G], F32)
                    nc.vector.reduce_max(out=m8[:], in_=u_sb[:], axis=X.X)
                    usq = work.tile([P, G, L], F32)
                    nc.vector.tensor_tensor(
                        out=usq[:],
                        in0=u_sb[:],
                        in1=u_sb[:],
                        op=mybir.AluOpType.mult,
                    )
                    nc.vector.reduce_sum(
                        out=accq[:, g : g + 1], in_=usq[:], axis=X.XY
                    )
                    s2s = small.tile([P, G], F32)
                    nc.vector.tensor_tensor(
                        out=s2s[:],
                        in0=s8[:],
                        in1=s8[:],
                        op=mybir.AluOpType.mult,
                    )
                    nc.vector.reduce_sum(
                        out=accs2[:, g : g + 1], in_=s2s[:], axis=X.X
                    )
                    nc.vector.reduce_sum(
                        out=accm[:, g : g + 1], in_=m8[:], axis=X.X
                    )
            nc.sync.dma_start(out_d.ap()[0], accq[:])
            nc.sync.dma_start(out_d.ap()[1], accs2[:])
            nc.sync.dma_start(out_d.ap()[2], accm[:])
    _split_multi_waits(nc)
    return nc


def make_constants(tunnel_to_link, link_capacities, np_pe_dt=np.float32):
    t2l = np.asarray(tunnel_to_link).astype(np.int64).ravel()
    cap = np.asarray(link_capacities, dtype=np.float32).ravel()
    mask = np.zeros((P, NCH * L), dtype=np.float32)
    for t in range(T):
        c, r = divmod(t, P)
        mask[r, c * L + int(t2l[t])] = 1.0
    ident = np.eye(P, dtype=np.float32)
    rrep = np.broadcast_to(
        (1.0 / (cap + 1e-8)).astype(np.float32)[None, :], (P, L)
    ).copy()
    return mask.astype(np_pe_dt), ident.astype(np_pe_dt), rrep


def run_cores(nc, pred, dem, mask, ident, rrep, **kw):
    pred = np.ascontiguousarray(np.asarray(pred, dtype=np.float32))
    dem = np.ascontiguousarray(np.asarray(dem, dtype=np.float32))
    in_maps = []
    for i in range(N_CORES):
        in_maps.append(
            {
                "pred": pred[i * ROWS : (i + 1) * ROWS],
                "dem": dem[i * ROWS : (i + 1) * ROWS],
                "mask": mask,
                "ident": ident,
                "rrep": rrep,
            }
        )
    return run_bass_kernel_spmd(nc, in_maps, core_ids=list(range(N_CORES)), **kw)


def combine_partials(partials_list):
    q = s2 = m = 0.0
    for p in partials_list:
        p = np.asarray(p, dtype=np.float64)
        q += p[0].sum()
        s2 += p[1].sum()
        m += p[2].sum()
    var_mean = (q - s2 / L) / (L - 1) / B
    return var_mean + 0.5 * m / B


def kernel(pred_ratios, demands, tunnel_to_link, link_capacities):
    mask, ident, rrep = make_constants(tunnel_to_link, link_capacities)
    nc = build_kernel()
    res = run_cores(nc, pred_ratios, demands, mask, ident, rrep)
    loss = combine_partials([r["partials"] for r in res.results])
    return np.array(loss, dtype=np.float32)

